# revision 31
# baseline (speedup 1.0000x reference)
"""Causal self-attention (B=4, T=2048, D=1024, H=16) on 8 trn2 NeuronCores.

Sharding: Megatron-style tensor parallel over heads (TP=2) x data parallel
over batch (DP=4).  Core c handles batch c//2 and head-group c%2 (8 heads).
Each core computes its QKV projection slice, causal attention for its 8
heads, and a partial output projection; the host sums the two TP partials
per batch and adds b_proj.

v8 schedule: one software-pipelined instruction stream.
  - Score matmuls contract over the 64-deep head dim; the two heads of a
    pair run as two concurrent row-group matmuls (PE rows 0-63 / 64-127),
    so a pair costs one N-stream instead of two.
  - QKV projection for block b+1 and the output projection for block b-1
    are woven as filler chains between the S/exp/PV steps of block b.
  - The scalar engine binds late attention (exp volume grows with the
    causal span), so q-block 3's work is pulled forward: key blocks 0-1
    run during block 1's window, key block 2 plus diagonal tiles 12-14
    during block 2's, with PV partials spilled to SBUF accumulators.
    Block 3's own window only covers its last diagonal tile.
  - Causal masking: gpsimd affine_select zeroes exp output on diagonal
    blocks (one 3D-AP op covers both heads); the pulled-forward diagonal
    chains mask on the DVE instead (gpsimd is loaded in those windows).
  - Diagonal exps use one 3D-AP activation covering both heads' valid
    regions (halves the scalar-engine instruction count there).
  - 48 zero matmuls at kernel start warm the PE clock while the (few,
    consolidated) input DMAs stream in.
All matmuls run in fp16 (fp32 PSUM accumulation); softmax in fp32 on the
scalar engine (exp) / DVE (reciprocal).  Output partials are written fp16
(host accumulates in fp32).
"""
import sys

sys.path.insert(0, "/opt/trn_rl_repo")

from collections import deque

import numpy as np

import concourse.bass as bass
import concourse.tile as tile
from concourse import bacc, mybir
from concourse.bass_utils import run_bass_kernel_spmd

B, T, D, H = 4, 2048, 1024, 16
HD = 64            # head dim
HL = 8             # heads per core (TP=2)
DL = HL * HD       # 512 local qkv width
KCH = D // 128     # 8 contraction chunks for QKV
NQB = T // 512     # 4 query blocks of 512
F16 = mybir.dt.float16
F32 = mybir.dt.float32
DEBUG_DUMP = False
NEG = -1.0e30

_cache = {}


def _build():
    nc = bacc.Bacc("TRN2", target_bir_lowering=False, num_devices=8)

    xT = nc.dram_tensor("xT", [D, T], F16, kind="ExternalInput")
    wq = nc.dram_tensor("wq", [D, DL], F16, kind="ExternalInput")
    wk = nc.dram_tensor("wk", [D, DL], F16, kind="ExternalInput")
    bqk = nc.dram_tensor("bqk", [128, 2 * DL // 128], F32, kind="ExternalInput")
    wv = nc.dram_tensor("wv", [D, DL], F16, kind="ExternalInput")
    bv = nc.dram_tensor("bv", [1, DL], F32, kind="ExternalInput")
    wp = nc.dram_tensor("wp", [DL, D], F16, kind="ExternalInput")
    out = nc.dram_tensor("out", [T, D], F16, kind="ExternalOutput")

    with tile.TileContext(nc) as tc:
        with (
            tc.tile_pool(name="const", bufs=1) as const,
            tc.tile_pool(name="acts", bufs=1) as acts,
            tc.tile_pool(name="esb", bufs=3) as esb,
            tc.tile_pool(name="small", bufs=3) as small,
            tc.tile_pool(name="outp", bufs=3) as outp,
            tc.tile_pool(name="pss", bufs=2, space="PSUM") as pss,
            tc.tile_pool(name="psy", bufs=1, space="PSUM") as psy,
            tc.tile_pool(name="pw", bufs=2, space="PSUM") as pw,
        ):
            # ---- PE warm-up: keep the HAM busy while inputs stream in ----
            zw = const.tile([128, 128], F16, name="zw", tag="zw")
            nc.gpsimd.memset(zw, 0.0)
            for i in range(32):
                psz = pw.tile([128, 512], F32, name="pw", tag="pw")
                nc.tensor.matmul(psz[:, 0:128], zw, zw, start=True, stop=True)

            # ---- inputs: a few consolidated DMAs (3D APs) ----
            xb0_sb = const.tile([128, KCH * 512], F16, name="xb0", tag="xb0")
            xrest_sb = const.tile([128, KCH * 1536], F16, name="xrest",
                                  tag="xrest")
            wq_sb = const.tile([128, KCH * 512], F16, name="wq", tag="wq")
            wk_sb = const.tile([128, KCH * 512], F16, name="wk", tag="wk")
            wv_sb = const.tile([128, KCH * 512], F16, name="wv", tag="wv")
            wp_sb = const.tile([128, 4 * D], F16, name="wp", tag="wp")
            bqk_sb = const.tile([128, 2 * DL // 128], F32)
            bv_sb = const.tile([1, DL], F32)
            x3 = xT.rearrange("(k p) t -> p k t", p=128)
            q3 = wq.rearrange("(k p) n -> p k n", p=128)
            k3 = wk.rearrange("(k p) n -> p k n", p=128)
            v3 = wv.rearrange("(k p) n -> p k n", p=128)
            p3 = wp.rearrange("(c p) n -> p c n", p=128)
            nc.sync.dma_start(
                out=xb0_sb.rearrange("p (k t) -> p k t", k=KCH),
                in_=x3[:, :, 0:512])
            nc.sync.dma_start(
                out=wq_sb.rearrange("p (k n) -> p k n", k=KCH),
                in_=q3[:, :, :])
            nc.sync.dma_start(out=bqk_sb, in_=bqk[:, :])
            nc.sync.dma_start(out=bv_sb, in_=bv[:, :])
            nc.sync.dma_start(
                out=wk_sb.rearrange("p (k n) -> p k n", k=KCH),
                in_=k3[:, :, :])
            nc.sync.dma_start(
                out=wv_sb.rearrange("p (k n) -> p k n", k=KCH),
                in_=v3[:, :, :])
            nc.sync.dma_start(
                out=xrest_sb.rearrange("p (k t) -> p k t", k=KCH),
                in_=x3[:, :, 512:T])
            nc.sync.dma_start(
                out=wp_sb.rearrange("p (c n) -> p c n", c=4),
                in_=p3[:, :, :])
            bvb_sb = const.tile([128, DL], F32)
            nc.gpsimd.partition_broadcast(bvb_sb, bv_sb)

            def x_slice(k, c0, c1):
                """xT chunk k, token-columns [c0:c1)."""
                if c1 <= 512:
                    return xb0_sb[:, 512 * k + c0:512 * k + c1]
                return xrest_sb[:, 1536 * k + c0 - 512:1536 * k + c1 - 512]

            # ---- persistent activations ----
            # qT/kT tile c: partitions 0:64 = head 2c dims, 64:128 = head
            # 2c+1 dims; free dim = T.  yT same channel layout.
            qT_sb = [acts.tile([128, T], F16, name=f"qT{c}", tag=f"qT{c}")
                     for c in range(4)]
            kT_sb = [acts.tile([128, T], F16, name=f"kT{c}", tag=f"kT{c}")
                     for c in range(4)]
            vaug = [acts.tile([128, HL * (HD + 1)], F16, name=f"va{t}",
                              tag=f"va{t}") for t in range(T // 128)]
            yT_sb = [acts.tile([128, T], F16, name=f"yT{c}", tag=f"yT{c}")
                     for c in range(4)]
            # SBUF spill accumulators for q-blocks 2 and 3's early PV work
            acc3 = [[acts.tile([HD + 1, 512], F32, name=f"acc{c}{p}",
                               tag=f"acc{c}{p}") for p in range(2)]
                    for c in range(4)]
            acc2 = [[acts.tile([HD + 1, 512], F32, name=f"acb{c}{p}",
                               tag=f"acb{c}{p}") for p in range(2)]
                    for c in range(4)]

            # ---------- filler units ----------
            def qkv_unit(b, cc):
                """Projection chain for output chunk cc of block b.
                cc 0..3 -> qT[cc], cc 4..7 -> kT[cc-4]."""
                bs = slice(512 * b, 512 * (b + 1))
                ps = pw.tile([128, 512], F32, name="pw", tag="pw")
                wsrc = wq_sb if cc < 4 else wk_sb
                co = 128 * (cc % 4)
                for k in range(KCH):
                    nc.tensor.matmul(
                        ps,
                        wsrc[:, 512 * k + co:512 * k + co + 128],
                        x_slice(k, 512 * b, 512 * (b + 1)),
                        start=(k == 0), stop=(k == KCH - 1),
                    )
                dst = qT_sb[cc] if cc < 4 else kT_sb[cc - 4]
                nc.vector.tensor_scalar_add(
                    out=dst[:, bs], in0=ps, scalar1=bqk_sb[:, cc:cc + 1])

            def v_unit(b, t2):
                """v projection for 128-token tile 4*b+t2 (natural layout,
                with the ones column for the softmax denominator)."""
                t = 4 * b + t2
                ps = pw.tile([128, 512], F32, name="pw", tag="pw")
                for k in range(KCH):
                    nc.tensor.matmul(
                        ps,
                        x_slice(k, 128 * t, 128 * (t + 1)),
                        wv_sb[:, 512 * k:512 * (k + 1)],
                        start=(k == 0), stop=(k == KCH - 1),
                    )
                va3 = vaug[t].rearrange("p (h c) -> p h c", c=HD + 1)
                nc.vector.tensor_add(
                    va3[:, :, 0:HD],
                    ps.rearrange("p (h d) -> p h d", d=HD),
                    bvb_sb.rearrange("p (h d) -> p h d", d=HD),
                )
                nc.gpsimd.memset(va3[:, :, HD], 1.0)

            def op_unit(q0, tq):
                """Output projection for 128-token tile 4*q0+tq."""
                t = 4 * q0 + tq
                ob = outp.tile([128, D], F16, name="ob", tag="ob")
                for nb in range(D // 512):
                    ps = pw.tile([128, 512], F32, name="pw", tag="pw")
                    for c in range(DL // 128):
                        nc.tensor.matmul(
                            ps,
                            yT_sb[c][:, 128 * t:128 * (t + 1)],
                            wp_sb[:, 1024 * c + 512 * nb:
                                  1024 * c + 512 * (nb + 1)],
                            start=(c == 0), stop=(c == DL // 128 - 1),
                        )
                    nc.vector.tensor_copy(ob[:, 512 * nb:512 * (nb + 1)], ps)
                nc.sync.dma_start(out=out[128 * t:128 * (t + 1), :], in_=ob)

            def emit_unit(u):
                kind = u[0]
                if kind == "qkv":
                    qkv_unit(u[1], u[2])
                elif kind == "v":
                    v_unit(u[1], u[2])
                else:
                    op_unit(u[1], u[2])

            # ---------- attention chain ----------
            def attn_chain(q0, c, t0, t1, first, last, pacer):
                acc = acc3 if q0 == 3 else acc2
                """S/exp/PV for key tiles [t0, t1) of (q-block q0, head pair
                c).  first/last mark the accumulation batch boundaries;
                non-last batches spill to acc3, the last batch normalizes
                into yT (merging acc3 for q-block 3)."""
                qs_full = slice(512 * q0, 512 * (q0 + 1))
                ps_yA = psy.tile([HD + 1, 512], F32, name="psyA", tag="psyA")
                ps_yB = psy.tile([HD + 1, 512], F32, name="psyB", tag="psyB")
                pend = None
                for t in range(t0, t1):
                    m = t - 4 * q0
                    lo = 128 * m if m > 0 else 0
                    ks = slice(128 * t, 128 * (t + 1))
                    qs = slice(512 * q0 + lo, 512 * (q0 + 1))
                    ps_s = pss.tile([128, 1024], F32, name="psS", tag="psS")
                    nc.tensor.matmul(
                        ps_s[:, lo:512],
                        kT_sb[c][0:64, ks], qT_sb[c][0:64, qs],
                        start=True, stop=True,
                    )
                    nc.tensor.matmul(
                        ps_s[:, 512 + lo:1024],
                        kT_sb[c][64:128, ks], qT_sb[c][64:128, qs],
                        start=True, stop=True,
                    )
                    es = esb.tile([128, 1024], F16, name="es", tag="es")
                    if lo == 0:
                        nc.scalar.activation(
                            out=es[:, 0:1024], in_=ps_s[:, 0:1024],
                            func=mybir.ActivationFunctionType.Exp)
                    else:
                        nc.scalar.activation(
                            out=es[:, lo:512], in_=ps_s[:, lo:512],
                            func=mybir.ActivationFunctionType.Exp)
                        nc.scalar.activation(
                            out=es[:, 512 + lo:1024],
                            in_=ps_s[:, 512 + lo:1024],
                            func=mybir.ActivationFunctionType.Exp)
                    if m >= 0:
                        # causal mask: zero exp output where col < row in
                        # the 128x128 diagonal sub-block
                        for p in range(2):
                            dg = slice(512 * p + lo, 512 * p + lo + 128)
                            nc.gpsimd.affine_select(
                                out=es[:, dg], in_=es[:, dg],
                                pattern=[[1, 128]],
                                compare_op=mybir.AluOpType.is_ge,
                                fill=0.0,
                                base=0,
                                channel_multiplier=-1,
                            )
                    if pend is not None:
                        pes, pt, plo = pend
                        for p, psY in ((0, ps_yA), (1, ps_yB)):
                            h = 2 * c + p
                            nc.tensor.matmul(
                                psY[:, plo:512],
                                vaug[pt][:, (HD + 1) * h:(HD + 1) * (h + 1)],
                                pes[:, 512 * p + plo:512 * (p + 1)],
                                start=(pt == t0), stop=False)
                    pend = (es, t, lo)
                    pacer()
                pes, pt, plo = pend
                for p, psY in ((0, ps_yA), (1, ps_yB)):
                    h = 2 * c + p
                    nc.tensor.matmul(
                        psY[:, plo:512],
                        vaug[pt][:, (HD + 1) * h:(HD + 1) * (h + 1)],
                        pes[:, 512 * p + plo:512 * (p + 1)],
                        start=(pt == t0), stop=True)
                if not last:
                    # spill the PV partial to SBUF (also releases the banks)
                    for p, psY in ((0, ps_yA), (1, ps_yB)):
                        if first:
                            nc.vector.tensor_copy(acc[c][p], psY)
                        else:
                            nc.vector.tensor_add(acc[c][p], acc[c][p], psY)
                    return
                for p, psY in ((0, ps_yA), (1, ps_yB)):
                    if first:
                        ysrc = psY  # normalize straight off PSUM
                    else:
                        ysrc = small.tile([HD + 1, 512], F32, name="cp",
                                          tag="cp")
                        nc.vector.tensor_add(ysrc, psY, acc[c][p])
                    # custom-DVE recip can't read from a non-zero base
                    # partition on HW: stage the denominator at partition 0.
                    dn = small.tile([1, 512], F32, name="dn", tag="dn")
                    nc.vector.tensor_copy(dn, ysrc[HD:HD + 1, :])
                    rc = small.tile([1, 512], F32, name="rc", tag="rc")
                    nc.vector.reciprocal_approx_fast(rc, dn)
                    rb = small.tile([64, 512], F32, name="rb", tag="rb")
                    nc.gpsimd.partition_broadcast(rb, rc)
                    nc.vector.tensor_mul(
                        yT_sb[c][64 * p:64 * (p + 1), qs_full],
                        ysrc[0:HD, :],
                        rb,
                    )

            # ---------- prologue: QKV for block 0 ----------
            for cc in range(8):
                qkv_unit(0, cc)
            for t2 in range(4):
                v_unit(0, t2)

            # ---------- pipelined attention over query blocks ----------
            for q0 in range(NQB):
                fillers = deque()
                if q0 == 0:
                    # block-2 queries feed the pulled-forward chains below;
                    # interleave them with block 1's projections
                    for c2 in range(4):
                        fillers.append(("qkv", 2, c2))
                        fillers.append(("qkv", 1, c2))
                        fillers.append(("qkv", 1, 4 + c2))
                    for t2 in range(4):
                        fillers.append(("v", 1, t2))
                    for cc in range(4):
                        fillers.append(("qkv", 3, cc))
                elif q0 == 1:
                    for cc in range(4, 8):
                        fillers.append(("qkv", 2, cc))
                    for t2 in range(4):
                        fillers.append(("v", 2, t2))
                elif q0 == 2:
                    # the rest of block 3's projections (keys and values)
                    for cc in range(4, 8):
                        fillers.append(("qkv", 3, cc))
                    for t2 in range(4):
                        fillers.append(("v", 3, t2))
                if q0 > 0:
                    for tq in range(4):
                        fillers.append(("op", q0 - 1, tq))

                native = (4 * q0 + 4) if q0 < 3 else 4
                extra = {0: 4, 1: 12, 2: 4, 3: 0}[q0]
                if q0 == 2:
                    native = 4
                nsteps = 4 * (native + extra)
                nfill = len(fillers)
                state = {"acc": 0.0}

                def pacer():
                    state["acc"] += nfill / nsteps
                    while fillers and state["acc"] >= 1.0:
                        emit_unit(fillers.popleft())
                        state["acc"] -= 1.0

                for c in range(4):
                    if q0 == 0:
                        attn_chain(0, c, 0, 4, first=True, last=True,
                                   pacer=pacer)
                        attn_chain(2, c, 0, 4, first=True, last=False,
                                   pacer=pacer)
                    elif q0 == 1:
                        attn_chain(1, c, 0, 8, first=True, last=True,
                                   pacer=pacer)
                        attn_chain(3, c, 0, 8, first=True, last=False,
                                   pacer=pacer)
                        attn_chain(2, c, 4, 8, first=False, last=False,
                                   pacer=pacer)
                    elif q0 == 2:
                        attn_chain(2, c, 8, 12, first=False, last=True,
                                   pacer=pacer)
                        attn_chain(3, c, 8, 12, first=False, last=False,
                                   pacer=pacer)
                    else:
                        attn_chain(3, c, 12, 16, first=False, last=True,
                                   pacer=pacer)
                while fillers:
                    emit_unit(fillers.popleft())

            # ---------- epilogue: output projection for the last block ----
            for tq in range(4):
                op_unit(NQB - 1, tq)

            if DEBUG_DUMP:
                dq = nc.dram_tensor("dbg_qT", [512, T], F16,
                                    kind="ExternalOutput")
                dk = nc.dram_tensor("dbg_kT", [512, T], F16,
                                    kind="ExternalOutput")
                dy = nc.dram_tensor("dbg_yT", [512, T], F16,
                                    kind="ExternalOutput")
                for c in range(4):
                    nc.sync.dma_start(out=dq[128 * c:128 * (c + 1), :],
                                      in_=qT_sb[c])
                    nc.sync.dma_start(out=dk[128 * c:128 * (c + 1), :],
                                      in_=kT_sb[c])
                    nc.sync.dma_start(out=dy[128 * c:128 * (c + 1), :],
                                      in_=yT_sb[c])

    nc.finalize()
    return nc


def _enable_trace_hooks():
    """Inject antenv.axon_hooks + no-op artifact upload so that
    run_bass_kernel_spmd(trace=True) works under axon in this image."""
    import types
    import antenv

    if "antenv.axon_hooks" not in sys.modules:
        mod = types.ModuleType("antenv.axon_hooks")
        state = {"hook": None}
        mod.set_axon_ntff_profile_hook = lambda h: state.__setitem__("hook", h)
        mod.get_axon_ntff_profile_hook = lambda: state["hook"]
        sys.modules["antenv.axon_hooks"] = mod
        antenv.axon_hooks = mod
        from trn_agent_boot.trn_boot import _ntff_profile_via_ctypes

        mod.set_axon_ntff_profile_hook(
            _ntff_profile_via_ctypes("/opt/axon/libaxon_pjrt.so"))
    from concourse import bass_utils as bu

    bu.upload_artifacts = lambda tmpdir: str(tmpdir)


def kernel(x, w_attn, b_attn, w_proj, b_proj, _trace=False):
    x = np.asarray(x)
    w_attn = np.asarray(w_attn)
    b_attn = np.asarray(b_attn)
    w_proj = np.asarray(w_proj)
    b_proj = np.asarray(b_proj)

    if "nc" not in _cache:
        _cache["nc"] = _build()
    nc = _cache["nc"]

    scale = 1.0 / np.sqrt(HD)
    f16 = np.float16

    in_maps = []
    for core in range(8):
        b, hg = core // 2, core % 2
        qs = slice(hg * DL, (hg + 1) * DL)
        ks = slice(D + hg * DL, D + (hg + 1) * DL)
        vs = slice(2 * D + hg * DL, 2 * D + (hg + 1) * DL)
        bqk_host = np.concatenate(
            [b_attn[qs] * scale, b_attn[ks]]).astype(np.float32)
        in_maps.append({
            "xT": np.ascontiguousarray(x[b].T).astype(f16),
            "wq": np.ascontiguousarray(w_attn[:, qs] * scale).astype(f16),
            "wk": np.ascontiguousarray(w_attn[:, ks]).astype(f16),
            "bqk": np.ascontiguousarray(bqk_host.reshape(8, 128).T),
            "wv": np.ascontiguousarray(w_attn[:, vs]).astype(f16),
            "bv": np.ascontiguousarray(b_attn[vs][None, :]).astype(np.float32),
            "wp": np.ascontiguousarray(w_proj[hg * DL:(hg + 1) * DL, :]).astype(f16),
        })

    kwargs = {}
    if _trace:
        _enable_trace_hooks()
        kwargs = dict(trace=True, trace_cores=[0])
    res = run_bass_kernel_spmd(nc, in_maps, core_ids=list(range(8)), **kwargs)

    outp = np.empty((B, T, D), np.float32)
    for b in range(B):
        outp[b] = (np.asarray(res.results[2 * b]["out"], np.float32)
                   + np.asarray(res.results[2 * b + 1]["out"], np.float32))
    outp += b_proj.astype(np.float32)

    if _trace:
        print(f"HW exec time: {res.exec_time_ns} ns")
    return outp


# revision 32
# speedup vs baseline: 1.0039x; 1.0039x over previous
"""Causal self-attention (B=4, T=2048, D=1024, H=16) on 8 trn2 NeuronCores.

Sharding: Megatron-style tensor parallel over heads (TP=2) x data parallel
over batch (DP=4).  Core c handles batch c//2 and head-group c%2 (8 heads).
Each core computes its QKV projection slice, causal attention for its 8
heads, and a partial output projection; the host sums the two TP partials
per batch and adds b_proj.

v8 schedule: one software-pipelined instruction stream.
  - Score matmuls contract over the 64-deep head dim; the two heads of a
    pair run as two concurrent row-group matmuls (PE rows 0-63 / 64-127),
    so a pair costs one N-stream instead of two.
  - QKV projection for block b+1 and the output projection for block b-1
    are woven as filler chains between the S/exp/PV steps of block b.
  - The scalar engine binds late attention (exp volume grows with the
    causal span), so q-block 3's work is pulled forward: key blocks 0-1
    run during block 1's window, key block 2 plus diagonal tiles 12-14
    during block 2's, with PV partials spilled to SBUF accumulators.
    Block 3's own window only covers its last diagonal tile.
  - Causal masking: gpsimd affine_select zeroes exp output on diagonal
    blocks (one 3D-AP op covers both heads); the pulled-forward diagonal
    chains mask on the DVE instead (gpsimd is loaded in those windows).
  - Diagonal exps use one 3D-AP activation covering both heads' valid
    regions (halves the scalar-engine instruction count there).
  - 48 zero matmuls at kernel start warm the PE clock while the (few,
    consolidated) input DMAs stream in.
All matmuls run in fp16 (fp32 PSUM accumulation); softmax in fp32 on the
scalar engine (exp) / DVE (reciprocal).  Output partials are written fp16
(host accumulates in fp32).
"""
import sys

sys.path.insert(0, "/opt/trn_rl_repo")

from collections import deque

import numpy as np

import concourse.bass as bass
import concourse.tile as tile
from concourse import bacc, mybir
from concourse.bass_utils import run_bass_kernel_spmd

B, T, D, H = 4, 2048, 1024, 16
HD = 64            # head dim
HL = 8             # heads per core (TP=2)
DL = HL * HD       # 512 local qkv width
KCH = D // 128     # 8 contraction chunks for QKV
NQB = T // 512     # 4 query blocks of 512
F16 = mybir.dt.float16
F32 = mybir.dt.float32
DEBUG_DUMP = False
NEG = -1.0e30

_cache = {}


def _build():
    nc = bacc.Bacc("TRN2", target_bir_lowering=False, num_devices=8)

    xT = nc.dram_tensor("xT", [D, T], F16, kind="ExternalInput")
    wq = nc.dram_tensor("wq", [D, DL], F16, kind="ExternalInput")
    wk = nc.dram_tensor("wk", [D, DL], F16, kind="ExternalInput")
    bqk = nc.dram_tensor("bqk", [128, 2 * DL // 128], F32, kind="ExternalInput")
    wv = nc.dram_tensor("wv", [D, DL], F16, kind="ExternalInput")
    bv = nc.dram_tensor("bv", [1, DL], F32, kind="ExternalInput")
    wp = nc.dram_tensor("wp", [DL, D], F16, kind="ExternalInput")
    out = nc.dram_tensor("out", [T, D], F16, kind="ExternalOutput")

    with tile.TileContext(nc) as tc:
        with (
            tc.tile_pool(name="const", bufs=1) as const,
            tc.tile_pool(name="acts", bufs=1) as acts,
            tc.tile_pool(name="esb", bufs=3) as esb,
            tc.tile_pool(name="small", bufs=3) as small,
            tc.tile_pool(name="outp", bufs=3) as outp,
            tc.tile_pool(name="pss", bufs=2, space="PSUM") as pss,
            tc.tile_pool(name="psy", bufs=1, space="PSUM") as psy,
            tc.tile_pool(name="pw", bufs=2, space="PSUM") as pw,
        ):
            # ---- PE warm-up: keep the HAM busy while inputs stream in ----
            zw = const.tile([128, 128], F16, name="zw", tag="zw")
            nc.gpsimd.memset(zw, 0.0)
            for i in range(32):
                psz = pw.tile([128, 512], F32, name="pw", tag="pw")
                nc.tensor.matmul(psz[:, 0:128], zw, zw, start=True, stop=True)

            # ---- inputs: a few consolidated DMAs (3D APs) ----
            xb0_sb = const.tile([128, KCH * 512], F16, name="xb0", tag="xb0")
            xrest_sb = const.tile([128, KCH * 1536], F16, name="xrest",
                                  tag="xrest")
            wq_sb = const.tile([128, KCH * 512], F16, name="wq", tag="wq")
            wk_sb = const.tile([128, KCH * 512], F16, name="wk", tag="wk")
            wv_sb = const.tile([128, KCH * 512], F16, name="wv", tag="wv")
            wp_sb = const.tile([128, 4 * D], F16, name="wp", tag="wp")
            bqk_sb = const.tile([128, 2 * DL // 128], F32)
            bv_sb = const.tile([1, DL], F32)
            x3 = xT.rearrange("(k p) t -> p k t", p=128)
            q3 = wq.rearrange("(k p) n -> p k n", p=128)
            k3 = wk.rearrange("(k p) n -> p k n", p=128)
            v3 = wv.rearrange("(k p) n -> p k n", p=128)
            p3 = wp.rearrange("(c p) n -> p c n", p=128)
            nc.sync.dma_start(
                out=xb0_sb.rearrange("p (k t) -> p k t", k=KCH),
                in_=x3[:, :, 0:512])
            nc.sync.dma_start(
                out=wq_sb.rearrange("p (k n) -> p k n", k=KCH),
                in_=q3[:, :, :])
            nc.sync.dma_start(out=bqk_sb, in_=bqk[:, :])
            nc.sync.dma_start(out=bv_sb, in_=bv[:, :])
            nc.sync.dma_start(
                out=wk_sb.rearrange("p (k n) -> p k n", k=KCH),
                in_=k3[:, :, :])
            nc.sync.dma_start(
                out=wv_sb.rearrange("p (k n) -> p k n", k=KCH),
                in_=v3[:, :, :])
            nc.sync.dma_start(
                out=xrest_sb.rearrange("p (k t) -> p k t", k=KCH),
                in_=x3[:, :, 512:T])
            nc.sync.dma_start(
                out=wp_sb.rearrange("p (c n) -> p c n", c=4),
                in_=p3[:, :, :])
            bvb_sb = const.tile([128, DL], F32)
            nc.gpsimd.partition_broadcast(bvb_sb, bv_sb)

            def x_slice(k, c0, c1):
                """xT chunk k, token-columns [c0:c1)."""
                if c1 <= 512:
                    return xb0_sb[:, 512 * k + c0:512 * k + c1]
                return xrest_sb[:, 1536 * k + c0 - 512:1536 * k + c1 - 512]

            # ---- persistent activations ----
            # qT/kT tile c: partitions 0:64 = head 2c dims, 64:128 = head
            # 2c+1 dims; free dim = T.  yT same channel layout.
            qT_sb = [acts.tile([128, T], F16, name=f"qT{c}", tag=f"qT{c}")
                     for c in range(4)]
            kT_sb = [acts.tile([128, T], F16, name=f"kT{c}", tag=f"kT{c}")
                     for c in range(4)]
            vaug = [acts.tile([128, HL * (HD + 1)], F16, name=f"va{t}",
                              tag=f"va{t}") for t in range(T // 128)]
            yT_sb = [acts.tile([128, T], F16, name=f"yT{c}", tag=f"yT{c}")
                     for c in range(4)]
            # SBUF spill accumulators for q-block 3's early PV partials
            acc3 = [[acts.tile([HD + 1, 512], F32, name=f"acc{c}{p}",
                               tag=f"acc{c}{p}") for p in range(2)]
                    for c in range(4)]

            # ---------- filler units ----------
            def qkv_unit(b, cc):
                """Projection chain for output chunk cc of block b.
                cc 0..3 -> qT[cc], cc 4..7 -> kT[cc-4]."""
                bs = slice(512 * b, 512 * (b + 1))
                ps = pw.tile([128, 512], F32, name="pw", tag="pw")
                wsrc = wq_sb if cc < 4 else wk_sb
                co = 128 * (cc % 4)
                for k in range(KCH):
                    nc.tensor.matmul(
                        ps,
                        wsrc[:, 512 * k + co:512 * k + co + 128],
                        x_slice(k, 512 * b, 512 * (b + 1)),
                        start=(k == 0), stop=(k == KCH - 1),
                    )
                dst = qT_sb[cc] if cc < 4 else kT_sb[cc - 4]
                nc.vector.tensor_scalar_add(
                    out=dst[:, bs], in0=ps, scalar1=bqk_sb[:, cc:cc + 1])

            def v_unit(b, t2):
                """v projection for 128-token tile 4*b+t2 (natural layout,
                with the ones column for the softmax denominator)."""
                t = 4 * b + t2
                ps = pw.tile([128, 512], F32, name="pw", tag="pw")
                for k in range(KCH):
                    nc.tensor.matmul(
                        ps,
                        x_slice(k, 128 * t, 128 * (t + 1)),
                        wv_sb[:, 512 * k:512 * (k + 1)],
                        start=(k == 0), stop=(k == KCH - 1),
                    )
                va3 = vaug[t].rearrange("p (h c) -> p h c", c=HD + 1)
                nc.vector.tensor_add(
                    va3[:, :, 0:HD],
                    ps.rearrange("p (h d) -> p h d", d=HD),
                    bvb_sb.rearrange("p (h d) -> p h d", d=HD),
                )
                nc.gpsimd.memset(va3[:, :, HD], 1.0)

            def op_unit(q0, tq):
                """Output projection for 128-token tile 4*q0+tq."""
                t = 4 * q0 + tq
                ob = outp.tile([128, D], F16, name="ob", tag="ob")
                for nb in range(D // 512):
                    ps = pw.tile([128, 512], F32, name="pw", tag="pw")
                    for c in range(DL // 128):
                        nc.tensor.matmul(
                            ps,
                            yT_sb[c][:, 128 * t:128 * (t + 1)],
                            wp_sb[:, 1024 * c + 512 * nb:
                                  1024 * c + 512 * (nb + 1)],
                            start=(c == 0), stop=(c == DL // 128 - 1),
                        )
                    nc.vector.tensor_copy(ob[:, 512 * nb:512 * (nb + 1)], ps)
                nc.sync.dma_start(out=out[128 * t:128 * (t + 1), :], in_=ob)

            def emit_unit(u):
                kind = u[0]
                if kind == "qkv":
                    qkv_unit(u[1], u[2])
                elif kind == "v":
                    v_unit(u[1], u[2])
                else:
                    op_unit(u[1], u[2])

            # ---------- attention chain ----------
            def attn_chain(q0, c, t0, t1, first, last, pacer):
                acc = acc3
                """S/exp/PV for key tiles [t0, t1) of (q-block q0, head pair
                c).  first/last mark the accumulation batch boundaries;
                non-last batches spill to acc3, the last batch normalizes
                into yT (merging acc3 for q-block 3)."""
                qs_full = slice(512 * q0, 512 * (q0 + 1))
                ps_yA = psy.tile([HD + 1, 512], F32, name="psyA", tag="psyA")
                ps_yB = psy.tile([HD + 1, 512], F32, name="psyB", tag="psyB")
                pend = None
                for t in range(t0, t1):
                    m = t - 4 * q0
                    lo = 128 * m if m > 0 else 0
                    ks = slice(128 * t, 128 * (t + 1))
                    qs = slice(512 * q0 + lo, 512 * (q0 + 1))
                    ps_s = pss.tile([128, 1024], F32, name="psS", tag="psS")
                    nc.tensor.matmul(
                        ps_s[:, lo:512],
                        kT_sb[c][0:64, ks], qT_sb[c][0:64, qs],
                        start=True, stop=True,
                    )
                    nc.tensor.matmul(
                        ps_s[:, 512 + lo:1024],
                        kT_sb[c][64:128, ks], qT_sb[c][64:128, qs],
                        start=True, stop=True,
                    )
                    es = esb.tile([128, 1024], F16, name="es", tag="es")
                    if lo == 0:
                        nc.scalar.activation(
                            out=es[:, 0:1024], in_=ps_s[:, 0:1024],
                            func=mybir.ActivationFunctionType.Exp)
                    else:
                        nc.scalar.activation(
                            out=es[:, lo:512], in_=ps_s[:, lo:512],
                            func=mybir.ActivationFunctionType.Exp)
                        nc.scalar.activation(
                            out=es[:, 512 + lo:1024],
                            in_=ps_s[:, 512 + lo:1024],
                            func=mybir.ActivationFunctionType.Exp)
                    if m >= 0:
                        # causal mask: zero exp output where col < row in
                        # the 128x128 diagonal sub-block
                        for p in range(2):
                            dg = slice(512 * p + lo, 512 * p + lo + 128)
                            nc.gpsimd.affine_select(
                                out=es[:, dg], in_=es[:, dg],
                                pattern=[[1, 128]],
                                compare_op=mybir.AluOpType.is_ge,
                                fill=0.0,
                                base=0,
                                channel_multiplier=-1,
                            )
                    if pend is not None:
                        pes, pt, plo = pend
                        for p, psY in ((0, ps_yA), (1, ps_yB)):
                            h = 2 * c + p
                            nc.tensor.matmul(
                                psY[:, plo:512],
                                vaug[pt][:, (HD + 1) * h:(HD + 1) * (h + 1)],
                                pes[:, 512 * p + plo:512 * (p + 1)],
                                start=(pt == t0), stop=False)
                    pend = (es, t, lo)
                    pacer()
                pes, pt, plo = pend
                for p, psY in ((0, ps_yA), (1, ps_yB)):
                    h = 2 * c + p
                    nc.tensor.matmul(
                        psY[:, plo:512],
                        vaug[pt][:, (HD + 1) * h:(HD + 1) * (h + 1)],
                        pes[:, 512 * p + plo:512 * (p + 1)],
                        start=(pt == t0), stop=True)
                if not last:
                    # spill the PV partial to SBUF (also releases the banks)
                    for p, psY in ((0, ps_yA), (1, ps_yB)):
                        if first:
                            nc.vector.tensor_copy(acc[c][p], psY)
                        else:
                            nc.vector.tensor_add(acc[c][p], acc[c][p], psY)
                    return
                for p, psY in ((0, ps_yA), (1, ps_yB)):
                    if first:
                        ysrc = psY  # normalize straight off PSUM
                    else:
                        ysrc = small.tile([HD + 1, 512], F32, name="cp",
                                          tag="cp")
                        nc.vector.tensor_add(ysrc, psY, acc[c][p])
                    # custom-DVE recip can't read from a non-zero base
                    # partition on HW: stage the denominator at partition 0.
                    dn = small.tile([1, 512], F32, name="dn", tag="dn")
                    nc.vector.tensor_copy(dn, ysrc[HD:HD + 1, :])
                    rc = small.tile([1, 512], F32, name="rc", tag="rc")
                    nc.vector.reciprocal_approx_fast(rc, dn)
                    rb = small.tile([64, 512], F32, name="rb", tag="rb")
                    nc.gpsimd.partition_broadcast(rb, rc)
                    nc.vector.tensor_mul(
                        yT_sb[c][64 * p:64 * (p + 1), qs_full],
                        ysrc[0:HD, :],
                        rb,
                    )

            # ---------- prologue: QKV for block 0 ----------
            for cc in range(8):
                qkv_unit(0, cc)
            for t2 in range(4):
                v_unit(0, t2)

            # ---------- pipelined attention over query blocks ----------
            for q0 in range(NQB):
                fillers = deque()
                if q0 < 2:
                    for cc in range(8):
                        fillers.append(("qkv", q0 + 1, cc))
                    for t2 in range(4):
                        fillers.append(("v", q0 + 1, t2))
                if q0 == 0:
                    # block 3's queries are consumed early (its off-diagonal
                    # attention runs during blocks 1-2), so project them now
                    for cc in range(4):
                        fillers.append(("qkv", 3, cc))
                elif q0 == 2:
                    # the rest of block 3's projections (keys and values)
                    for cc in range(4, 8):
                        fillers.append(("qkv", 3, cc))
                    for t2 in range(4):
                        fillers.append(("v", 3, t2))
                if q0 > 0:
                    for tq in range(4):
                        fillers.append(("op", q0 - 1, tq))

                native = (4 * q0 + 4) if q0 < 3 else 4
                extra = 8 if q0 == 1 else (4 if q0 == 2 else 0)
                nsteps = 4 * (native + extra)
                nfill = len(fillers)
                state = {"acc": 0.0}

                def pacer():
                    state["acc"] += nfill / nsteps
                    while fillers and state["acc"] >= 1.0:
                        emit_unit(fillers.popleft())
                        state["acc"] -= 1.0

                for c in range(4):
                    if q0 < 3:
                        attn_chain(q0, c, 0, 4 * q0 + 4, first=True,
                                   last=True, pacer=pacer)
                    else:
                        attn_chain(3, c, 12, 16, first=False, last=True,
                                   pacer=pacer)
                    if q0 == 1:
                        attn_chain(3, c, 0, 8, first=True, last=False,
                                   pacer=pacer)
                    elif q0 == 2:
                        attn_chain(3, c, 8, 12, first=False, last=False,
                                   pacer=pacer)
                while fillers:
                    emit_unit(fillers.popleft())

            # ---------- epilogue: output projection for the last block ----
            for tq in range(4):
                op_unit(NQB - 1, tq)

            if DEBUG_DUMP:
                dq = nc.dram_tensor("dbg_qT", [512, T], F16,
                                    kind="ExternalOutput")
                dk = nc.dram_tensor("dbg_kT", [512, T], F16,
                                    kind="ExternalOutput")
                dy = nc.dram_tensor("dbg_yT", [512, T], F16,
                                    kind="ExternalOutput")
                for c in range(4):
                    nc.sync.dma_start(out=dq[128 * c:128 * (c + 1), :],
                                      in_=qT_sb[c])
                    nc.sync.dma_start(out=dk[128 * c:128 * (c + 1), :],
                                      in_=kT_sb[c])
                    nc.sync.dma_start(out=dy[128 * c:128 * (c + 1), :],
                                      in_=yT_sb[c])

    nc.finalize()
    return nc


def _enable_trace_hooks():
    """Inject antenv.axon_hooks + no-op artifact upload so that
    run_bass_kernel_spmd(trace=True) works under axon in this image."""
    import types
    import antenv

    if "antenv.axon_hooks" not in sys.modules:
        mod = types.ModuleType("antenv.axon_hooks")
        state = {"hook": None}
        mod.set_axon_ntff_profile_hook = lambda h: state.__setitem__("hook", h)
        mod.get_axon_ntff_profile_hook = lambda: state["hook"]
        sys.modules["antenv.axon_hooks"] = mod
        antenv.axon_hooks = mod
        from trn_agent_boot.trn_boot import _ntff_profile_via_ctypes

        mod.set_axon_ntff_profile_hook(
            _ntff_profile_via_ctypes("/opt/axon/libaxon_pjrt.so"))
    from concourse import bass_utils as bu

    bu.upload_artifacts = lambda tmpdir: str(tmpdir)


def kernel(x, w_attn, b_attn, w_proj, b_proj, _trace=False):
    x = np.asarray(x)
    w_attn = np.asarray(w_attn)
    b_attn = np.asarray(b_attn)
    w_proj = np.asarray(w_proj)
    b_proj = np.asarray(b_proj)

    if "nc" not in _cache:
        _cache["nc"] = _build()
    nc = _cache["nc"]

    scale = 1.0 / np.sqrt(HD)
    f16 = np.float16

    in_maps = []
    for core in range(8):
        b, hg = core // 2, core % 2
        qs = slice(hg * DL, (hg + 1) * DL)
        ks = slice(D + hg * DL, D + (hg + 1) * DL)
        vs = slice(2 * D + hg * DL, 2 * D + (hg + 1) * DL)
        bqk_host = np.concatenate(
            [b_attn[qs] * scale, b_attn[ks]]).astype(np.float32)
        in_maps.append({
            "xT": np.ascontiguousarray(x[b].T).astype(f16),
            "wq": np.ascontiguousarray(w_attn[:, qs] * scale).astype(f16),
            "wk": np.ascontiguousarray(w_attn[:, ks]).astype(f16),
            "bqk": np.ascontiguousarray(bqk_host.reshape(8, 128).T),
            "wv": np.ascontiguousarray(w_attn[:, vs]).astype(f16),
            "bv": np.ascontiguousarray(b_attn[vs][None, :]).astype(np.float32),
            "wp": np.ascontiguousarray(w_proj[hg * DL:(hg + 1) * DL, :]).astype(f16),
        })

    kwargs = {}
    if _trace:
        _enable_trace_hooks()
        kwargs = dict(trace=True, trace_cores=[0])
    res = run_bass_kernel_spmd(nc, in_maps, core_ids=list(range(8)), **kwargs)

    outp = np.empty((B, T, D), np.float32)
    for b in range(B):
        outp[b] = (np.asarray(res.results[2 * b]["out"], np.float32)
                   + np.asarray(res.results[2 * b + 1]["out"], np.float32))
    outp += b_proj.astype(np.float32)

    if _trace:
        print(f"HW exec time: {res.exec_time_ns} ns")
    return outp
